# revision 1
# baseline (speedup 1.0000x reference)
"""Trainium2 Bass kernel for the low-rank MGD (Mahalanobis Gaussian) loss.

Strategy (data-parallel over batch across 8 NeuronCores):
  - Each core receives a [384, 4000] shard of x (384 = 16 samples x 24
    q-rows) and computes, fully on device, its samples' Mahalanobis
    ingredients: per-row sums of x^2 (fused DVE multiply-reduce) and
    z^T[j, (s,i)] = sum_{n,q} x[(s,q),n] Lq_s[q,i] Ln_s[n,j] via two
    PSUM-accumulated matmul stages (x as the bf16 stationary operand
    against a block-diagonal Lq_s, then Ln_s^T against the stage-1
    output). No transposes are needed anywhere.
  - The y_t != 0 mask is handled on the host: y_t is randn-filled, so it
    contains an exact f32 zero with probability ~0; kernel() verifies that
    and falls back to masking x on the host in the degenerate case. The
    device therefore only streams x (49MB instead of 98MB).
  - Host gathers the tiny per-core outputs (z [B, 360] and row sums) and
    finishes: the 360x360 capacitance cholesky / logdet / triangular
    solve, and the final scalar loss. This is ~30 MFLOP of O(R^3) linear
    algebra on 47KB of data - negligible next to what the device streams.
"""

import os
import sys
import types
from contextlib import ExitStack

import numpy as np

if "/opt/trn_rl_repo" not in sys.path:
    sys.path.insert(0, "/opt/trn_rl_repo")

import concourse.bass as bass
import concourse.tile as tile
import concourse.mybir as mybir
from concourse.bass_utils import run_bass_kernel_spmd
from concourse.vector_clock import ScopedClock

F32 = mybir.dt.float32

# Problem constants (hardcoded per the harness contract).
B, Q, N = 128, 24, 4000
RANK_N, RANK_Q = 30, 12
SIGMA_INIT = 1.0
SIGMA_MIN = 0.001
NCORES = 8
BSH = B // NCORES          # samples per core = 16
ROWS = BSH * Q             # (b, q) rows per core = 384
RT = ROWS // 128           # 128-row tiles per core = 3
NCH = 32                   # matmul n-chunks of 128 (last 32)
CH = 128
# chunks per phase: small first (fast pipeline fill), big in the middle
# (few triggers at steady state), small last (fast drain)
PH = [2, 6, 8, 8, 4, 2, 1, 1]
NPH = len(PH)
PH_OFF = [sum(PH[:i]) for i in range(NPH)]      # first chunk of each phase

LAST_EXEC_TIME_NS = None


# ---------------------------------------------------------------------------
# Environment fixups
# ---------------------------------------------------------------------------

_MAX_WAITS = 1  # walrus codegen here rejects multiple sync-waits on one instruction


def _apply_tile_wait_split_patch():
    """walrus in this image rejects >2 sync-waits on one instruction
    ("Too many sync wait commands"). Split excess waits onto same-engine
    nops placed immediately before the over-subscribed instruction, and
    do the same for the Tile tail Drain."""
    if getattr(tile.TileContext, "_wait_split_applied", False):
        return

    orig_lower = tile.TileContext._lower_ordered_insts

    def _split_waits(self, ordered):
        for bb_name, insts in ordered.items():
            out = []
            for inst in insts:
                si = inst.sync_info
                if si is not None and len(si.on_wait) > _MAX_WAITS:
                    waits = list(si.on_wait)
                    rest, keep = waits[:-_MAX_WAITS], waits[-_MAX_WAITS:]
                    inst.sync_info = mybir.SyncInfo(
                        on_update=list(si.on_update), on_wait=keep
                    )
                    for i in range(0, len(rest), _MAX_WAITS):
                        out.append(
                            mybir.InstNoOp(
                                name=f"{inst.name}.wsplit{i}",
                                engine=inst.engine,
                                bass_nofuse=True,
                                sync_info=mybir.SyncInfo(
                                    on_update=[],
                                    on_wait=rest[i : i + _MAX_WAITS],
                                ),
                            )
                        )
                out.append(inst)
            ordered[bb_name] = out

    def _lower_ordered_insts(self, ordered):
        _split_waits(self, ordered)
        return orig_lower(self, ordered)

    def _drain_and_barrier(self, tick_clock, wait_clock):
        drain_inst = self.nc.sync.drain()
        wait_clock.add_sem_waits(
            drain_inst.ins, ScopedClock({None: tick_clock.global_clock})
        )
        waits = list(drain_inst.ins.sync_info.on_wait)
        if len(waits) > _MAX_WAITS:
            drain_inst.ins.sync_info.on_wait = waits[:_MAX_WAITS]
            rest = waits[_MAX_WAITS:]
            for i in range(0, len(rest), _MAX_WAITS):
                nop = self.nc.sync.nop(nofuse=True, hint="drain_wait_split")
                nop.ins.sync_info = mybir.SyncInfo(
                    on_update=[], on_wait=rest[i : i + _MAX_WAITS]
                )

        tail_mode = os.environ.get("BASS_TAIL_MODE", "slim")
        assert self.sems is not None
        popped = self.nc._tile_sem_poison_stack.pop()
        assert popped is self._sem_poison
        if tail_mode == "full":
            self.nc.all_engine_barrier()
            self.nc.clear_and_free_semaphores(list(self.sems.allocated().values()))
            self.nc.all_engine_barrier()
        elif tail_mode == "slim":
            # Engine streams end right after the clear; the next execute
            # of this NEFF can only be submitted after every stream (incl.
            # gpsimd's clears) has retired, so the trailing barrier is
            # redundant for a non-looping kernel.
            self.nc.all_engine_barrier()
            self.nc.clear_and_free_semaphores(list(self.sems.allocated().values()))
        elif tail_mode == "semonly":
            self.nc.all_engine_barrier(sem_only=True)
            self.nc.clear_and_free_semaphores(list(self.sems.allocated().values()))
        elif tail_mode == "none":
            pass  # drain only; relies on NRT resetting sem state per execute
        else:
            raise ValueError(f"unknown BASS_TAIL_MODE {tail_mode}")

    tile.TileContext._lower_ordered_insts = _lower_ordered_insts
    tile.TileContext._drain_and_barrier = _drain_and_barrier
    tile.TileContext._wait_split_applied = True


def _install_ntff_hook():
    """Register the axon NTFF profile hook (the image's antenv package lacks
    axon_hooks, so trace=True would silently degrade otherwise)."""
    if "antenv.axon_hooks" in sys.modules:
        return
    mod = types.ModuleType("antenv.axon_hooks")
    state = {"hook": None}
    mod.set_axon_ntff_profile_hook = lambda h: state.__setitem__("hook", h)
    mod.get_axon_ntff_profile_hook = lambda: state["hook"]
    sys.modules["antenv.axon_hooks"] = mod
    try:
        import antenv

        antenv.axon_hooks = mod
    except Exception:
        pass
    try:
        from trn_agent_boot.trn_boot import _ntff_profile_via_ctypes

        hook = _ntff_profile_via_ctypes("/opt/axon/libaxon_pjrt.so")
        if hook is not None:
            mod.set_axon_ntff_profile_hook(hook)
    except Exception:
        pass


_apply_tile_wait_split_patch()
_install_ntff_hook()


# ---------------------------------------------------------------------------
# Device kernel
# ---------------------------------------------------------------------------

ZW = BSH * RANK_Q          # z^T columns per core = 192
BF16 = mybir.dt.bfloat16


def _chunk_cols(c):
    return min(CH, N - CH * c)


def _phase_cols(p):
    return sum(_chunk_cols(PH_OFF[p] + i) for i in range(PH[p]))


def _build_nc():
    """Per core: z^T = sum_n sum_q x[(s,q), n] Lq_s[q, i] Ln_s[n, j].

    Stage 1 (per n-chunk c of 128): psum_T[n', (s,i)] accumulates
    x_tile_r^T @ BD_r over the 3 row-tiles r, where BD_r is the
    block-diagonal Lq_s for the samples covered by rows [128r, 128r+128).
    Samples straddling a row-tile boundary are summed by the PSUM
    accumulation. x tiles are the stationary operand in natural layout
    (no transposes; every PE op is a real matmul), converted to bf16 so
    LDWEIGHTS runs with FWL and overlaps in-flight matmuls.

    Stage 2: psum_z[j, (s,i)] accumulates lns_c^T @ T_c over the 32
    chunks. Stage-2 matmuls are emitted DELAY chunks behind stage 1 so
    the PSUM->SBUF copy of T_c is off the PE critical path.
    """
    nc = bass.Bass()
    x = nc.declare_dram_parameter("x", [ROWS, N], F32, isOutput=False)
    lns = nc.declare_dram_parameter("lns", [128, NCH * RANK_N], BF16, isOutput=False)
    bd = nc.declare_dram_parameter("bd", [128, RT * ZW], BF16, isOutput=False)
    zt = nc.declare_dram_parameter("zt", [RANK_N, ZW], F32, isOutput=True)
    rs = nc.declare_dram_parameter("rs", [128, RT * NPH], F32, isOutput=True)

    mult = mybir.AluOpType.mult
    DELAY = 4
    MAXPC = max(PH) * CH   # largest phase width in columns

    with tile.TileContext(nc) as tc, ExitStack() as ctx:
        const = ctx.enter_context(tc.tile_pool(name="const", bufs=1))
        bfp = [
            ctx.enter_context(tc.tile_pool(name=f"bf{r}", bufs=NPH))
            for r in range(RT)
        ]
        sqp = ctx.enter_context(tc.tile_pool(name="sq", bufs=3))
        ttp = ctx.enter_context(tc.tile_pool(name="tt", bufs=DELAY + 2))
        outp = ctx.enter_context(tc.tile_pool(name="outs", bufs=1))
        pt = ctx.enter_context(tc.tile_pool(name="pt", bufs=DELAY + 2, space="PSUM"))
        pz = ctx.enter_context(tc.tile_pool(name="pz", bufs=1, space="PSUM"))

        rs_sb = outp.tile([128, RT * NPH], F32)
        pzt = pz.tile([RANK_N, ZW], F32)
        pending = []  # (chunk, tt tile) awaiting the stage-2 matmul

        def stage2(c, tt):
            csz = _chunk_cols(c)
            nc.tensor.matmul(
                pzt[:],
                lns_sb[0:csz, RANK_N * c : RANK_N * (c + 1)],
                tt[0:csz, :],
                start=(c == 0),
                stop=(c == NCH - 1),
            )

        # Persistent bf16 image of x, one tile per 128-row tile; DMA phases
        # write column slices so triggers never wait on buffer recycling.
        # Phase-0 x loads go out first; constants follow (they are only
        # needed once the first matmuls run).
        bd_sb = const.tile([128, RT * ZW], BF16)
        lns_sb = const.tile([128, NCH * RANK_N], BF16)
        xbf = [[None] * NPH for _ in range(RT)]
        for r in range(RT):
            xb = bfp[r].tile([128, MAXPC], BF16, name=f"xb{r}_0", tag=f"xb{r}")
            nc.gpsimd.dma_start(
                xb[0:128, 0 : _phase_cols(0)],
                x[128 * r : 128 * (r + 1), 0 : _phase_cols(0)],
            )
            xbf[r][0] = xb
        nc.sync.dma_start(bd_sb[:], bd[:])
        nc.sync.dma_start(lns_sb[:], lns[:])

        # Warmup matmuls on constants: keep the PE busy through the DMA
        # ramp so the HAM clock gate opens (1.2 -> 2.4 GHz) before the
        # real matmuls start.
        n_warm = int(os.environ.get("BASS_WARM_MM", "24"))
        if n_warm:
            pj = pz.tile([128, 512], F32, tag="junk")
            for _ in range(n_warm):
                nc.tensor.matmul(
                    pj[:], bd_sb[:, 0:128], bd_sb[:, 0:512], start=True, stop=True
                )

        for p in range(NPH):
            pcols = _phase_cols(p)
            col0 = CH * PH_OFF[p]
            for r in range(RT):
                if p > 0:
                    # Casting DMA (SWDGE): f32 DRAM -> bf16 SBUF.
                    xb = bfp[r].tile([128, MAXPC], BF16, name=f"xb{r}_{p}", tag=f"xb{r}")
                    nc.gpsimd.dma_start(
                        xb[0:128, 0:pcols],
                        x[128 * r : 128 * (r + 1), col0 : col0 + pcols],
                    )
                    xbf[r][p] = xb
                xb = xbf[r][p]
                # x^2 (to scratch) + rowsum accumulator in one DVE op
                sq = sqp.tile([128, MAXPC], BF16)
                slot = r * NPH + p
                nc.vector.scalar_tensor_tensor(
                    sq[0:128, 0:pcols],
                    xb[0:128, 0:pcols],
                    1.0,
                    xb[0:128, 0:pcols],
                    mult,
                    mult,
                    accum_out=rs_sb[:, slot : slot + 1],
                )
            for cc in range(PH[p]):
                c = PH_OFF[p] + cc
                csz = _chunk_cols(c)
                ptc = pt.tile([CH, ZW], F32)
                for r in range(RT):
                    nc.tensor.matmul(
                        ptc[0:csz, :],
                        xbf[r][p][:, CH * cc : CH * cc + csz],
                        bd_sb[:, ZW * r : ZW * (r + 1)],
                        start=(r == 0),
                        stop=(r == RT - 1),
                    )
                tt = ttp.tile([CH, ZW], BF16)
                # PSUM->SBUF copies on ScalarE (otherwise mostly idle).
                nc.scalar.copy(tt[0:csz, :], ptc[0:csz, :])
                pending.append((c, tt))
                if len(pending) > DELAY:
                    stage2(*pending.pop(0))
        for c, tt in pending:
            stage2(c, tt)

        zto = outp.tile([RANK_N, ZW], F32, tag="zto")
        nc.scalar.copy(zto[:], pzt[:])
        nc.sync.dma_start(zt[:], zto[:])
        # Copy through DVE (program order after all accum writers) so the
        # DMA-out has a tracked producer for every element.
        rs_out = outp.tile([128, RT * NPH], F32, tag="rs_out")
        nc.vector.tensor_copy(rs_out[:], rs_sb[:])
        nc.sync.dma_start(rs[:], rs_out[:])
    return nc


_NC = None


def _get_nc():
    global _NC
    if _NC is None:
        _NC = _build_nc()
    return _NC


# ---------------------------------------------------------------------------
# Host wrapper
# ---------------------------------------------------------------------------

def kernel(eps_t, y_t, L_n, L_q, sigma):
    global LAST_EXEC_TIME_NS
    eps_t = np.ascontiguousarray(eps_t, dtype=np.float32)
    y_t = np.ascontiguousarray(y_t, dtype=np.float32)
    L_n = np.asarray(L_n, dtype=np.float32)
    L_q = np.asarray(L_q, dtype=np.float32)
    sigma = np.asarray(sigma, dtype=np.float32)
    assert eps_t.shape == (B, Q, N) and y_t.shape == (B, Q, N)

    import ml_dtypes

    lns = np.ascontiguousarray(L_n / np.float32(np.sqrt(RANK_N)))
    lqs32 = (L_q / np.float32(np.sqrt(RANK_Q))).astype(np.float32)
    lqs = lqs32.astype(np.float64)

    # lns row-packed into chunks of 128: lnp[p, 30c + j] = lns[128c + p, j]
    lnp = np.zeros((128, NCH * RANK_N), dtype=np.float32)
    for c in range(NCH):
        csz = _chunk_cols(c)
        lnp[:csz, RANK_N * c : RANK_N * (c + 1)] = lns[CH * c : CH * c + csz]
    lnp = lnp.astype(ml_dtypes.bfloat16)

    # Block-diagonal Lq_s per 128-row tile: bd[p, r*ZW + s*12 + i] =
    # lqs[q, i] where 128r + p = 24s + q (sample-local rows).
    bdm = np.zeros((128, RT * ZW), dtype=np.float32)
    for r in range(RT):
        for p in range(128):
            g = 128 * r + p
            s, q = divmod(g, Q)
            bdm[p, r * ZW + s * RANK_Q : r * ZW + (s + 1) * RANK_Q] = lqs32[q]
    bdm = bdm.astype(ml_dtypes.bfloat16)

    # The reference masks x where y_t is exactly 0.0f. y_t is randn-filled,
    # so this never fires in practice; handle the degenerate case on the
    # host so the device only has to stream x.
    if np.any(y_t == 0.0):
        eps_t = eps_t * (y_t != 0.0).astype(np.float32)

    xf = eps_t.reshape(B * Q, N)
    in_maps = [
        {
            "x": np.ascontiguousarray(xf[i * ROWS : (i + 1) * ROWS]),
            "lns": lnp,
            "bd": bdm,
        }
        for i in range(NCORES)
    ]

    nc = _get_nc()
    trace = bool(os.environ.get("BASS_KERNEL_TRACE"))
    res = run_bass_kernel_spmd(nc, in_maps, list(range(NCORES)), trace=trace)
    if trace:
        LAST_EXEC_TIME_NS = res.exec_time_ns

    # Gather z [B, R] (device zt is [30, (s, i)] per core) and row sums.
    z = np.concatenate(
        [
            res.results[i]["zt"]
            .astype(np.float64)
            .reshape(RANK_N, BSH, RANK_Q)
            .transpose(1, 2, 0)
            .reshape(BSH, RANK_Q * RANK_N)
            for i in range(NCORES)
        ]
    )
    rows = np.concatenate(
        [
            res.results[i]["rs"].reshape(128, RT, NPH).sum(axis=2).T.reshape(ROWS)
            for i in range(NCORES)
        ]
    )

    return _host_finish(z, rows, lqs, lns.astype(np.float64), sigma)


def _host_finish(z, rows, lqs, lns64, sigma):
    """Tiny O(R^3) finish in float64. z: [B, R]; rows: [B*Q] sums of
    masked x^2; lqs/lns64: scaled cov factors in float64."""
    D = Q * N
    R = RANK_Q * RANK_N

    s2 = rows.astype(np.float64).reshape(B, Q).sum(axis=1)

    # Capacitance grams: A = lqs^T lqs (rq x rq), Bm = lns^T lns (rn x rn).
    A = lqs.T @ lqs
    Bm = lns64.T @ lns64

    diag_bias = np.log(np.expm1(np.float64(SIGMA_INIT**2)))
    c = np.logaddexp(0.0, np.float64(sigma[0]) + diag_bias) + SIGMA_MIN**2

    cap = np.eye(R) + np.kron(A, Bm) / c
    L = np.linalg.cholesky(cap)
    logdet = 2.0 * np.sum(np.log(np.diagonal(L))) + D * np.log(c)

    try:
        from scipy.linalg import solve_triangular

        u = solve_triangular(L, z.T, lower=True)
    except Exception:
        u = np.linalg.solve(L, z.T)
    maha = s2 / c - (u * u).sum(axis=0) / (c * c)

    loss = np.mean(0.5 * (D * np.log(2.0 * np.pi) + logdet + maha))
    return np.float32(loss)



# revision 4
# speedup vs baseline: 1.5461x; 1.5461x over previous
"""Trainium2 Bass kernel for the low-rank MGD (Mahalanobis Gaussian) loss.

Strategy (data-parallel over batch across 8 NeuronCores):
  - Host packs each core's x shard (384 rows x 4000 cols) as a transposed
    fp8-e4m3 "SBUF image" xt[128, 32*384]: column block c holds n-chunk
    [128c, 128c+128) of x^T, so a straight 2D DMA lands it matmul-ready.
    fp8 quarters the HBM traffic vs the f32 baseline (1.6MB/core) and the
    2e-2 rel-err gate has ~70x margin (measured 2.8e-4 in numpy).
  - The contraction t[j, row] = sum_n lns[n, j] x[row, n] runs n-chunk by
    n-chunk with the 32-wide (30 Ln cols + 2 zero pad) stationary operand
    column-tiled into the four 32-col strips of the PE array: chunks 4g+s
    accumulate into PSUM partitions [32s, 32s+32), and the four strips'
    matmuls execute concurrently (xbus per col-group). The host sums the
    four strips - the device ships one [128, 384] bf16 tile.
  - ||x||^2 row sums only enter the loss as a per-core total, so each
    engine (DVE, ACT, GPSIMD) squares+row-sum-accumulates a slice of each
    DMA phase; the host reduces the tiny accumulator tile.
  - Host finishes: z = Lq-contraction of t (1M MACs), the 360x360
    capacitance cholesky / logdet / triangular solve, final scalar loss.
  - The y_t != 0 mask is handled on the host: y_t is randn-filled, so it
    contains an exact f32 zero with probability ~0; kernel() verifies that
    and falls back to masking x on the host in the degenerate case.
"""

import os
import sys
import types
from contextlib import ExitStack

import numpy as np

if "/opt/trn_rl_repo" not in sys.path:
    sys.path.insert(0, "/opt/trn_rl_repo")

import concourse.bass as bass
import concourse.tile as tile
import concourse.mybir as mybir
from concourse.bass_utils import run_bass_kernel_spmd
from concourse.vector_clock import ScopedClock

F32 = mybir.dt.float32
BF16 = mybir.dt.bfloat16
FP8 = mybir.dt.float8e4

# Problem constants (hardcoded per the harness contract).
B, Q, N = 128, 24, 4000
RANK_N, RANK_Q = 30, 12
SIGMA_INIT = 1.0
SIGMA_MIN = 0.001
NCORES = 8
BSH = B // NCORES          # samples per core = 16
ROWS = BSH * Q             # (b, q) rows per core = 384
NCH = 32                   # n-chunks of 128 (4000 zero-padded to 4096)
NPAD = NCH * 128
LNW = 32                   # stationary width per chunk (30 + 2 zero pad)

# DMA phases in chunks (each chunk = 384 fp8 cols = 48KB): small first for
# a fast pipeline fill, small last for a fast drain.
PHASES = [2, 4, 4, 6, 6, 4, 4, 2]
assert sum(PHASES) == NCH
PH_OFF = [sum(PHASES[:i]) for i in range(len(PHASES))]

# Square-accumulate work split: (engine, phase, lo_chunk, hi_chunk) with
# chunk indices local to the phase. Rates ~ DVE 0.96 (fp8 runs the 1x DVE
# mode), ACT 1.2 cols/ns; the Pool engine has no elementwise ISA op on
# TRN2, so only D/A split the work.
SQ_TABLE = [
    ("D", 0, 0, 2),
    ("A", 1, 0, 4),
    ("D", 2, 0, 4),
    ("A", 3, 0, 4),
    ("D", 3, 4, 6),
    ("A", 4, 0, 4),
    ("D", 4, 4, 6),
    ("A", 5, 0, 2),
    ("D", 5, 2, 4),
    ("A", 6, 0, 3),
    ("D", 6, 3, 4),
    ("A", 7, 0, 1),
    ("D", 7, 1, 2),
]


def _check_sq_table():
    cover = set()
    for _, p, lo, hi in SQ_TABLE:
        assert 0 <= lo < hi <= PHASES[p]
        for c in range(PH_OFF[p] + lo, PH_OFF[p] + hi):
            assert c not in cover
            cover.add(c)
    assert cover == set(range(NCH))


_check_sq_table()

LAST_EXEC_TIME_NS = None


# ---------------------------------------------------------------------------
# Environment fixups
# ---------------------------------------------------------------------------

_MAX_WAITS = 1  # walrus codegen here rejects multiple sync-waits on one instruction


def _apply_tile_wait_split_patch():
    """walrus in this image rejects >2 sync-waits on one instruction
    ("Too many sync wait commands"). Split excess waits onto same-engine
    nops placed immediately before the over-subscribed instruction, and
    do the same for the Tile tail Drain."""
    if getattr(tile.TileContext, "_wait_split_applied", False):
        return

    orig_lower = tile.TileContext._lower_ordered_insts

    def _split_waits(self, ordered):
        for bb_name, insts in ordered.items():
            out = []
            for inst in insts:
                si = inst.sync_info
                if si is not None and len(si.on_wait) > _MAX_WAITS:
                    waits = list(si.on_wait)
                    rest, keep = waits[:-_MAX_WAITS], waits[-_MAX_WAITS:]
                    inst.sync_info = mybir.SyncInfo(
                        on_update=list(si.on_update), on_wait=keep
                    )
                    for i in range(0, len(rest), _MAX_WAITS):
                        out.append(
                            mybir.InstNoOp(
                                name=f"{inst.name}.wsplit{i}",
                                engine=inst.engine,
                                bass_nofuse=True,
                                sync_info=mybir.SyncInfo(
                                    on_update=[],
                                    on_wait=rest[i : i + _MAX_WAITS],
                                ),
                            )
                        )
                out.append(inst)
            ordered[bb_name] = out

    def _lower_ordered_insts(self, ordered):
        _split_waits(self, ordered)
        return orig_lower(self, ordered)

    def _drain_and_barrier(self, tick_clock, wait_clock):
        drain_inst = self.nc.sync.drain()
        wait_clock.add_sem_waits(
            drain_inst.ins, ScopedClock({None: tick_clock.global_clock})
        )
        waits = list(drain_inst.ins.sync_info.on_wait)
        if len(waits) > _MAX_WAITS:
            drain_inst.ins.sync_info.on_wait = waits[:_MAX_WAITS]
            rest = waits[_MAX_WAITS:]
            for i in range(0, len(rest), _MAX_WAITS):
                nop = self.nc.sync.nop(nofuse=True, hint="drain_wait_split")
                nop.ins.sync_info = mybir.SyncInfo(
                    on_update=[], on_wait=rest[i : i + _MAX_WAITS]
                )

        tail_mode = os.environ.get("BASS_TAIL_MODE", "slim")
        assert self.sems is not None
        popped = self.nc._tile_sem_poison_stack.pop()
        assert popped is self._sem_poison
        if tail_mode == "full":
            self.nc.all_engine_barrier()
            self.nc.clear_and_free_semaphores(list(self.sems.allocated().values()))
            self.nc.all_engine_barrier()
        elif tail_mode == "slim":
            # Engine streams end right after the clear; the next execute
            # of this NEFF can only be submitted after every stream (incl.
            # gpsimd's clears) has retired, so the trailing barrier is
            # redundant for a non-looping kernel.
            self.nc.all_engine_barrier()
            self.nc.clear_and_free_semaphores(list(self.sems.allocated().values()))
        elif tail_mode == "semonly":
            self.nc.all_engine_barrier(sem_only=True)
            self.nc.clear_and_free_semaphores(list(self.sems.allocated().values()))
        elif tail_mode == "none":
            pass  # drain only; relies on NRT resetting sem state per execute
        else:
            raise ValueError(f"unknown BASS_TAIL_MODE {tail_mode}")

    tile.TileContext._lower_ordered_insts = _lower_ordered_insts
    tile.TileContext._drain_and_barrier = _drain_and_barrier
    tile.TileContext._wait_split_applied = True


def _install_ntff_hook():
    """Register the axon NTFF profile hook (the image's antenv package lacks
    axon_hooks, so trace=True would silently degrade otherwise)."""
    if "antenv.axon_hooks" in sys.modules:
        return
    mod = types.ModuleType("antenv.axon_hooks")
    state = {"hook": None}
    mod.set_axon_ntff_profile_hook = lambda h: state.__setitem__("hook", h)
    mod.get_axon_ntff_profile_hook = lambda: state["hook"]
    sys.modules["antenv.axon_hooks"] = mod
    try:
        import antenv

        antenv.axon_hooks = mod
    except Exception:
        pass
    try:
        from trn_agent_boot.trn_boot import _ntff_profile_via_ctypes

        hook = _ntff_profile_via_ctypes("/opt/axon/libaxon_pjrt.so")
        if hook is not None:
            mod.set_axon_ntff_profile_hook(hook)
    except Exception:
        pass


_apply_tile_wait_split_patch()
_install_ntff_hook()


# ---------------------------------------------------------------------------
# Device kernel
# ---------------------------------------------------------------------------

NSQ = len(SQ_TABLE)


def _build_nc():
    """Per core: tT[j, row] += sum over n-chunks of lnp_c^T @ xt_c, with the
    32 chunks column-tiled 4-wide across the PE array (chunk 4g+s -> PSUM
    partitions [32s, 32s+32)), plus per-engine square+row-sum accumulation
    of every x element. Outputs: the raw 4-strip [128, 384] bf16 tT tile
    (host sums strips) and the [128, NSQ] f32 rowsum accumulators."""
    nc = bass.Bass()
    xt = nc.declare_dram_parameter("xt", [128, NCH * ROWS], FP8, isOutput=False)
    lnp = nc.declare_dram_parameter("lnp", [128, NCH * LNW], FP8, isOutput=False)
    tt = nc.declare_dram_parameter("tt", [128, ROWS], BF16, isOutput=True)
    rs = nc.declare_dram_parameter("rs", [128, NSQ], F32, isOutput=True)

    mult = mybir.AluOpType.mult
    SQF = mybir.ActivationFunctionType.Square
    maxw = max(PHASES) * ROWS
    nph = len(PHASES)

    with tile.TileContext(nc) as tc, ExitStack() as ctx:
        const = ctx.enter_context(tc.tile_pool(name="const", bufs=1))
        xpool = ctx.enter_context(tc.tile_pool(name="xph", bufs=nph))
        sqp = {
            e: ctx.enter_context(tc.tile_pool(name=f"sq{e}", bufs=2))
            for e in ("D", "A", "G")
        }
        outp = ctx.enter_context(tc.tile_pool(name="outs", bufs=1))
        pt = ctx.enter_context(tc.tile_pool(name="pt", bufs=1, space="PSUM"))

        # Constants: lnp on the ACT HWDGE ring (off the x stream's ring).
        lnp_sb = const.tile([128, NCH * LNW], FP8)
        nc.scalar.dma_start(lnp_sb[:], lnp[:])

        # x phases on the SP HWDGE ring, issued up front in arrival order.
        xph = []
        for p, nch in enumerate(PHASES):
            xb = xpool.tile([128, maxw], FP8, name=f"xph{p}", tag="xph")
            pc = nch * ROWS
            c0 = PH_OFF[p] * ROWS
            nc.sync.dma_start(xb[0:128, 0:pc], xt[0:128, c0 : c0 + pc])
            xph.append(xb)

        psum = pt.tile([128, ROWS], F32)
        rs_t = {}
        for e in ("D", "A", "G"):
            n = sum(1 for t in SQ_TABLE if t[0] == e)
            rs_t[e] = outp.tile(
                [128, max(n, 1)], F32, name=f"rs{e}", tag=f"rs{e}"
            )
        rs_out = outp.tile([128, NSQ], F32, tag="rs_out")

        def chunk_phase(c):
            for p in range(nph):
                if PH_OFF[p] <= c < PH_OFF[p] + PHASES[p]:
                    return p, c - PH_OFF[p]
            raise AssertionError

        # Square + row-sum accumulate, split across engines per SQ_TABLE.
        # Emitted before the matmul loop so each engine's program order
        # matches phase arrival order.
        slot = {"D": 0, "A": 0, "G": 0}
        for e, p, lo, hi in SQ_TABLE:
            xs = xph[p][0:128, lo * ROWS : hi * ROWS]
            sq = sqp[e].tile([128, maxw], BF16, tag=f"sq{e}")
            acc = rs_t[e][:, slot[e] : slot[e] + 1]
            slot[e] += 1
            if e == "A":
                nc.scalar.activation(
                    sq[0:128, 0 : (hi - lo) * ROWS], xs, SQF, accum_out=acc
                )
            else:
                eng = nc.vector if e == "D" else nc.gpsimd
                eng.scalar_tensor_tensor(
                    sq[0:128, 0 : (hi - lo) * ROWS], xs, 1.0, xs, mult, mult,
                    accum_out=acc,
                )

        # Column-tiled matmul groups: chunk 4g+s -> PSUM strip s.
        for g in range(NCH // 4):
            for s in range(4):
                c = 4 * g + s
                p, cl = chunk_phase(c)
                nc.tensor.matmul(
                    psum[32 * s : 32 * s + 32, :],
                    lnp_sb[:, LNW * c : LNW * (c + 1)],
                    xph[p][:, cl * ROWS : (cl + 1) * ROWS],
                    start=(g == 0),
                    stop=(g == NCH // 4 - 1),
                    tile_position=(0, 32 * s),
                )

        # Ship the raw 4-strip PSUM tile (host sums strips); every row is
        # written (rows 30-31 of each strip come from the zero-pad weight
        # columns), so one full-tile copy is safe.
        tto = outp.tile([128, ROWS], BF16, tag="tto")
        nc.scalar.copy(tto[:], psum[:])
        nc.sync.dma_start(tt[:], tto[:])

        # Each engine copies its own accumulator slots (creates a tracked
        # producer on the writing engine), then one DMA out.
        off = 0
        for e in ("D", "A", "G"):
            n = slot[e]
            if n == 0:
                continue
            dst = rs_out[:, off : off + n]
            src = rs_t[e][:, 0:n]
            if e == "A":
                nc.scalar.copy(dst, src)
            elif e == "D":
                nc.vector.tensor_copy(dst, src)
            else:
                nc.gpsimd.tensor_copy(dst, src)
            off += n
        nc.scalar.dma_start(rs[:], rs_out[:])
    return nc


_NC = None


def _get_nc():
    global _NC
    if _NC is None:
        _NC = _build_nc()
    return _NC


# ---------------------------------------------------------------------------
# Host wrapper
# ---------------------------------------------------------------------------

def kernel(eps_t, y_t, L_n, L_q, sigma):
    global LAST_EXEC_TIME_NS
    eps_t = np.ascontiguousarray(eps_t, dtype=np.float32)
    y_t = np.ascontiguousarray(y_t, dtype=np.float32)
    L_n = np.asarray(L_n, dtype=np.float32)
    L_q = np.asarray(L_q, dtype=np.float32)
    sigma = np.asarray(sigma, dtype=np.float32)
    assert eps_t.shape == (B, Q, N) and y_t.shape == (B, Q, N)

    import ml_dtypes

    lns = np.ascontiguousarray(L_n / np.float32(np.sqrt(RANK_N)))
    lqs = (L_q / np.float32(np.sqrt(RANK_Q))).astype(np.float64)

    # lnp[p, 32c + j] = lns[128c + p, j], j < 30; zero-padded n rows and
    # two zero j columns per chunk block.
    lnp = np.zeros((NPAD, LNW), dtype=np.float32)
    lnp[:N, :RANK_N] = lns
    lnp = np.ascontiguousarray(
        lnp.reshape(NCH, 128, LNW).transpose(1, 0, 2).reshape(128, NCH * LNW)
    ).astype(ml_dtypes.float8_e4m3)

    # The reference masks x where y_t is exactly 0.0f. y_t is randn-filled,
    # so this never fires in practice; handle the degenerate case on the
    # host so the device only has to stream x.
    if np.any(y_t == 0.0):
        eps_t = eps_t * (y_t != 0.0).astype(np.float32)

    # Per-core transposed fp8 image: xt[p, 384c + r] = x_core[r, 128c + p].
    xf = eps_t.reshape(B * Q, N)
    x8 = np.zeros((B * Q, NPAD), dtype=ml_dtypes.float8_e4m3)
    x8[:, :N] = xf.astype(ml_dtypes.float8_e4m3)
    in_maps = []
    for i in range(NCORES):
        sh = x8[i * ROWS : (i + 1) * ROWS]  # [384, 4096]
        img = np.ascontiguousarray(
            sh.reshape(ROWS, NCH, 128).transpose(2, 1, 0).reshape(128, NCH * ROWS)
        )
        in_maps.append({"xt": img, "lnp": lnp})

    nc = _get_nc()
    trace = bool(os.environ.get("BASS_KERNEL_TRACE"))
    res = run_bass_kernel_spmd(nc, in_maps, list(range(NCORES)), trace=trace)
    if trace:
        LAST_EXEC_TIME_NS = res.exec_time_ns

    # Gather: sum the 4 strips of tt -> tT [30, 384] per core, then the tiny
    # q-contraction z[s,i,j] = sum_q lqs[q,i] t[(s,q), j] in f64.
    z_parts = []
    s2 = 0.0
    for i in range(NCORES):
        ttc = res.results[i]["tt"].astype(np.float64)  # [128, 384]
        tT = ttc.reshape(4, 32, ROWS)[:, :RANK_N, :].sum(axis=0)  # [30, 384]
        t = tT.T.reshape(BSH, Q, RANK_N)
        z_parts.append(
            np.einsum("qi,sqj->sij", lqs, t).reshape(BSH, RANK_Q * RANK_N)
        )
        s2 += float(res.results[i]["rs"].astype(np.float64).sum())
    z = np.concatenate(z_parts)

    return _host_finish(z, s2, lqs, lns.astype(np.float64), sigma)


def _host_finish(z, s2, lqs, lns64, sigma):
    """Tiny O(R^3) finish in float64. z: [B, R]; s2: total sum of masked
    x^2; lqs/lns64: scaled cov factors in float64."""
    D = Q * N
    R = RANK_Q * RANK_N

    A = lqs.T @ lqs
    Bm = lns64.T @ lns64

    diag_bias = np.log(np.expm1(np.float64(SIGMA_INIT**2)))
    c = np.logaddexp(0.0, np.float64(sigma[0]) + diag_bias) + SIGMA_MIN**2

    cap = np.eye(R) + np.kron(A, Bm) / c
    L = np.linalg.cholesky(cap)
    logdet = 2.0 * np.sum(np.log(np.diagonal(L))) + D * np.log(c)

    try:
        from scipy.linalg import solve_triangular

        u = solve_triangular(L, z.T, lower=True)
    except Exception:
        u = np.linalg.solve(L, z.T)
    maha = s2 / B / c - (u * u).sum(axis=0).mean() / (c * c)

    loss = 0.5 * (D * np.log(2.0 * np.pi) + logdet + maha)
    return np.float32(loss)
